# revision 7
# baseline (speedup 1.0000x reference)
"""Trainium2 Bass kernel for nn_Attentionv2 (B=8, N=1024, C=768, H=12, D=64).

Strategy: data-parallel over batch — one batch element per NeuronCore (8 cores).
Per core, multi-head attention is computed entirely in the "transposed"
orientation so no on-chip transposes are needed:

  QT[h*64+d, n] = sum_c WqT[c, h*64+d] * xT[c, n]     (head-pair tiles)
  KT likewise; V[n, h*64+d] = sum_c xT[c, n-tile] * WvT[c, :]
  ST[m, n]  = sum_d KT[d, m] * QT[d, n]               (scores transposed;
               the two heads of a pair sit on partitions 0-63 / 64-127 so
               their K=64 matmuls dual-stream on the two PE row groups)
  ET        = exp(ST * 1/8)                            (no max-subtraction:
                                                        scores are O(1) here)
  PV lhsT   = [V_h | ones(64 cols)]  =>  out rows 0-63 = OT_h (unnorm),
               rows 64-127 = softmax denominator replicated 64x
  OT_norm   = OT * recip(Z)
  y[n, o]   = sum_c OT_norm[c, n] * WpT[c, o] + bp[o]

Pipelining: the kernel is paced by the ACT engine (96 exp calls ~110us).
Every score step (one (j,mt) pair: 2 dual-stream MMs + 2 exps) is followed
by exactly one filler group on the PE — V-projection t-tiles during pair 0,
then PV(j-1) quarter-groups and QK(j+1) quarter-groups — so neither PE nor
ACT ever starves and HAM stays at full clock.  Input DMAs are split into
prioritized chunks (x halves + w chunk-groups on 4 queues) so the first
exp issues ~11us in instead of ~56us.  One PSUM pool with 3 tags
(s:4 banks, qk:2, o:2) covers all phases — the output projection reuses
the score banks, avoiding a pool-boundary drain.

Matmul operands are fp16 (full-rate PE); accumulation is fp32 in PSUM.
"""

import numpy as np

P = 128
B, N, C = 8, 1024, 768
H, D = 12, 64
SCALE = D ** -0.5  # 0.125
CT = C // P   # 6 contraction chunks
NT = N // P   # 8 sequence tiles
HP = H // 2   # 6 head pairs
NCORES = 8

_cache = {}


def _build_nc():
    import concourse.bass as bass
    import concourse.mybir as mybir
    import concourse.tile as tile
    from concourse import bacc

    f32 = mybir.dt.float32
    f16 = mybir.dt.float16
    Exp = mybir.ActivationFunctionType.Exp

    nc = bacc.Bacc("TRN2", target_bir_lowering=False, debug=False,
                   enable_asserts=False)

    xT = nc.dram_tensor("xT", [C, N], f16, kind="ExternalInput").ap()
    wqT = nc.dram_tensor("wqT", [C, H * D], f16, kind="ExternalInput").ap()
    wkT = nc.dram_tensor("wkT", [C, H * D], f16, kind="ExternalInput").ap()
    wvT = nc.dram_tensor("wvT", [C, H * D], f16, kind="ExternalInput").ap()
    wpT = nc.dram_tensor("wpT", [C, C], f16, kind="ExternalInput").ap()
    bpb = nc.dram_tensor("bpb", [P, C], f32, kind="ExternalInput").ap()
    y = nc.dram_tensor("y", [N, C], f32, kind="ExternalOutput").ap()

    mm = nc.tensor.matmul

    xTr = xT.rearrange("(o p) n -> p o n", p=P)
    wqTr = wqT.rearrange("(o p) f -> p o f", p=P)
    wkTr = wkT.rearrange("(o p) f -> p o f", p=P)
    wvTr = wvT.rearrange("(o p) f -> p o f", p=P)
    wpTr = wpT.rearrange("(o p) f -> p o f", p=P)

    with tile.TileContext(nc) as tc:
        with tc.tile_pool(name="persist", bufs=1) as persist, \
             tc.tile_pool(name="ph1", bufs=1) as ph1, \
             tc.tile_pool(name="ps", bufs=1, space="PSUM") as psp, \
             tc.tile_pool(name="et", bufs=24) as etp, \
             tc.tile_pool(name="sm", bufs=4) as smp, \
             tc.tile_pool(name="outp", bufs=3) as outp:
            qt = persist.tile([P, HP, N], f16)
            kt = persist.tile([P, HP, N], f16)
            vp = persist.tile([P, NT, H, 2 * D], f16)  # [Vh | ones]
            ot = persist.tile([P, HP, N], f16)
            wp_sb = persist.tile([P, CT, C], f16)
            bpb_sb = persist.tile([P, C], f32)

            x_sb = ph1.tile([P, CT, N], f16)
            wq_sb = ph1.tile([P, CT, H * D], f16)
            wk_sb = ph1.tile([P, CT, H * D], f16)
            wv_sb = ph1.tile([P, CT, H * D], f16)

            # --- prioritized chunked input DMAs on 4 queues.  x halves +
            # wq/wk chunk-groups first (QK(0) gates the first exp), wv next
            # (needed by the V-proj fillers from ~13us), wp/bpb last. ---
            nc.sync.dma_start(x_sb[:, :, 0:512], xTr[:, :, 0:512])
            nc.scalar.dma_start(wq_sb[:, 0:3, :], wqTr[:, 0:3, :])
            nc.gpsimd.dma_start(wk_sb[:, 0:3, :], wkTr[:, 0:3, :])
            nc.sync.dma_start(x_sb[:, :, 512:1024], xTr[:, :, 512:1024])
            nc.scalar.dma_start(wq_sb[:, 3:6, :], wqTr[:, 3:6, :])
            nc.gpsimd.dma_start(wv_sb[:, 0:3, :], wvTr[:, 0:3, :])
            nc.gpsimd.dma_start(wk_sb[:, 3:6, :], wkTr[:, 3:6, :])
            nc.gpsimd.dma_start(wv_sb[:, 3:6, :], wvTr[:, 3:6, :])
            nc.gpsimd.dma_start(wp_sb[:], wpTr[:])
            nc.scalar.dma_start(bpb_sb[:], bpb)

            nc.vector.memset(vp[:, :, :, D:2 * D], 1.0)

            def emit_qk_group(j, gi):
                # gi 0..3 = (q,nh0), (k,nh0), (q,nh1), (k,nh1)
                w_sb, dst = ((wq_sb, qt), (wk_sb, kt))[gi % 2]
                nh = gi // 2
                ps = psp.tile([P, 512], f32, tag="qk", name="qkps")
                for c in range(CT):
                    mm(ps[:], lhsT=w_sb[:, c, j * P:(j + 1) * P],
                       rhs=x_sb[:, c, nh * 512:(nh + 1) * 512],
                       start=(c == 0), stop=(c == CT - 1))
                nc.vector.tensor_copy(
                    dst[:, j, nh * 512:(nh + 1) * 512], ps[:])

            def emit_v_t(t):
                psa = psp.tile([P, 512], f32, tag="qk", name="psa")
                psb = psp.tile([P, 512], f32, tag="qk", name="psb")
                for c in range(CT):
                    lh = x_sb[:, c, t * P:(t + 1) * P]
                    mm(psa[:], lhsT=lh, rhs=wv_sb[:, c, 0:512],
                       start=(c == 0), stop=(c == CT - 1))
                    mm(psb[:, 0:256], lhsT=lh, rhs=wv_sb[:, c, 512:768],
                       start=(c == 0), stop=(c == CT - 1))
                nc.vector.tensor_copy(
                    vp[:, t, 0:8, 0:D],
                    psa.rearrange("p (h d) -> p h d", d=D))
                nc.vector.tensor_copy(
                    vp[:, t, 8:12, 0:D],
                    psb[:, 0:256].rearrange("p (h d) -> p h d", d=D))

            ets = {}

            def emit_scores_mt(j, mt):
                s = {}
                for hh in range(2):
                    s[hh] = psp.tile([P, N], f32, tag="s", name=f"s_{hh}")
                    ets[(j, hh, mt)] = etp.tile([P, N], f16, tag="et",
                                                name=f"et_{hh}")
                for nh in range(2):
                    for hh in range(2):   # adjacent => PE row-group dual
                        r0 = hh * D
                        mm(s[hh][:, nh * 512:(nh + 1) * 512],
                           lhsT=kt[r0:r0 + D, j, mt * P:(mt + 1) * P],
                           rhs=qt[r0:r0 + D, j, nh * 512:(nh + 1) * 512],
                           start=True, stop=True)
                for hh in range(2):
                    nc.scalar.activation(ets[(j, hh, mt)][:], s[hh][:],
                                         Exp, scale=float(SCALE))

            def emit_pv_group(j, hh, nh):
                h = 2 * j + hh
                r0 = hh * D
                pso = psp.tile([P, 512], f32, tag="o", name="pso")
                for mt in range(NT):
                    mm(pso[:], lhsT=vp[:, mt, h],
                       rhs=ets[(j, hh, mt)][:, nh * 512:(nh + 1) * 512],
                       start=(mt == 0), stop=(mt == NT - 1))
                sums = smp.tile([D, 512], f32, tag="sums")
                rec = smp.tile([D, 512], f32, tag="rec")
                nc.vector.tensor_copy(sums[:], pso[D:2 * D, :])
                nc.vector.reciprocal_approx_fast(rec[:], sums[:])
                nc.vector.tensor_mul(
                    ot[r0:r0 + D, j, nh * 512:(nh + 1) * 512],
                    pso[0:D, :], rec[:])

            # ---- prologue: QK for pair 0 (DMA-paced) ----
            for gi in range(4):
                emit_qk_group(0, gi)

            # ---- filler queue: one PE group per score step, in dependency
            # order.  QK(j+1) groups (q=12 MMs, k=12 MMs) always land within
            # pair j's steps; V tiles precede PV(0); PV(j) follows et(j). ----
            # Slot constraints (PE queue is in-order, so violating either
            # deadlocks or stalls the queue):
            #  - pv(jj) group slots must be >= 8*(jj+1): only after pair
            #    jj's own score steps are all emitted may PV(jj) wait on
            #    its exps, else scores behind it starve ACT (circular wait).
            #  - qk(j) group slots must be < 8*j (scores(j,0) reads qt/kt).
            fillers = [None] * (HP * NT)
            fillers[0] = ("qk", 1, 0)
            fillers[1] = ("qk", 1, 1)
            for t in range(NT):                       # slots 2-9
                fillers[2 + t] = ("v", t)
            for jj in range(5):                       # pv(jj): slots 8jj+10
                for g in range(4):
                    fillers[8 * jj + 10 + g] = ("pv", jj, g // 2, g % 2)
            for j in range(2, HP):                    # qk(j): slots 8j-2
                fillers[8 * j - 2] = ("qk", j, 0)
                fillers[8 * j - 1] = ("qk", j, 1)
            for s, f in enumerate(fillers):
                if f and f[0] == "pv":
                    assert s >= 8 * (f[1] + 1), (s, f)
                if f and f[0] == "qk":
                    assert s < 8 * f[1], (s, f)

            def emit_filler(f):
                if f is None:
                    return
                if f[0] == "qk":
                    _, j, half = f
                    emit_qk_group(j, 2 * half)      # (q or k, nh0)
                    emit_qk_group(j, 2 * half + 1)  # (q or k, nh1)
                elif f[0] == "v":
                    emit_v_t(f[1])
                else:
                    _, j, hh, nh = f
                    emit_pv_group(j, hh, nh)

            step = 0
            for j in range(HP):
                for mt in range(NT):
                    emit_scores_mt(j, mt)
                    emit_filler(fillers[step])
                    step += 1

            # ---- tail: PV of the last pair, then output projection ----
            for hh in range(2):
                for nh in range(2):
                    emit_pv_group(HP - 1, hh, nh)

            yre = y.rearrange("(t p) f -> t p f", p=P)
            for t in range(NT):
                yps = psp.tile([P, C], f32, tag="s", name="yps")
                for c in range(CT):
                    lh = ot[:, c, t * P:(t + 1) * P]
                    mm(yps[:, 0:512], lhsT=lh, rhs=wp_sb[:, c, 0:512],
                       start=(c == 0), stop=(c == CT - 1))
                    mm(yps[:, 512:768], lhsT=lh, rhs=wp_sb[:, c, 512:768],
                       start=(c == 0), stop=(c == CT - 1))
                ys = outp.tile([P, C], f32, tag="ys")
                nc.vector.tensor_add(ys[:, 0:512], yps[:, 0:512],
                                     bpb_sb[:, 0:512])
                nc.vector.tensor_add(ys[:, 512:768], yps[:, 512:768],
                                     bpb_sb[:, 512:768])
                eng = nc.sync if t % 2 == 0 else nc.scalar
                eng.dma_start(yre[t], ys[:])

    nc.compile()
    return nc


def _get_nc():
    if "nc" not in _cache:
        _cache["nc"] = _build_nc()
    return _cache["nc"]


def _make_in_maps(x, Wq, Wk, Wv, Wp, bp):
    x = np.asarray(x, dtype=np.float32)
    wqT = np.ascontiguousarray(
        np.asarray(Wq, np.float32).reshape(H * D, C).T.astype(np.float16))
    wkT = np.ascontiguousarray(
        np.asarray(Wk, np.float32).reshape(H * D, C).T.astype(np.float16))
    wvT = np.ascontiguousarray(
        np.asarray(Wv, np.float32).reshape(H * D, C).T.astype(np.float16))
    wpT = np.ascontiguousarray(
        np.asarray(Wp, np.float32).T.astype(np.float16))
    bpb = np.ascontiguousarray(
        np.broadcast_to(np.asarray(bp, np.float32), (P, C)))
    in_maps = []
    for b in range(NCORES):
        in_maps.append({
            "xT": np.ascontiguousarray(x[b].T.astype(np.float16)),
            "wqT": wqT, "wkT": wkT, "wvT": wvT, "wpT": wpT, "bpb": bpb,
        })
    return in_maps


def run(x, Wq, Wk, Wv, Wp, bp, trace=False):
    from concourse.bass_utils import run_bass_kernel_spmd
    nc = _get_nc()
    in_maps = _make_in_maps(x, Wq, Wk, Wv, Wp, bp)
    res = run_bass_kernel_spmd(nc, in_maps, list(range(NCORES)), trace=trace)
    out = np.stack([res.results[b]["y"] for b in range(NCORES)])
    return out, res


def kernel(x, Wq, Wk, Wv, Wp, bp):
    out, _ = run(x, Wq, Wk, Wv, Wp, bp)
    return out


# revision 11
# speedup vs baseline: 1.1036x; 1.1036x over previous
"""Trainium2 Bass kernel for nn_Attentionv2 (B=8, N=1024, C=768, H=12, D=64).

Strategy: data-parallel over batch — one batch element per NeuronCore (8 cores).
Per core, multi-head attention is computed entirely in the "transposed"
orientation so no on-chip transposes are needed:

  QT[h*64+d, n] = sum_c WqT[c, h*64+d] * xT[c, n]     (head-pair tiles)
  KT likewise; V[n, h*64+d] = sum_c xT[c, n-tile] * WvT[c, :]
  ST[m, n]  = sum_d KT[d, m] * QT[d, n]               (scores transposed;
               the two heads of a pair sit on partitions 0-63 / 64-127 so
               their K=64 matmuls dual-stream on the two PE row groups)
  ET        = exp(ST * 1/8)                            (no max-subtraction:
                                                        scores are O(1) here)
  PV lhsT   = [V_h | ones(64 cols)]  =>  out rows 0-63 = OT_h (unnorm),
               rows 64-127 = softmax denominator replicated 64x
  OT_norm   = OT * recip(Z)
  y[n, o]   = sum_c OT_norm[c, n] * WpT[c, o] + bp[o]

Pipelining: the kernel is paced by the ACT engine (96 exp calls ~110us).
Every score step (one (j,mt) pair: 2 dual-stream MMs + 2 exps) is followed
by exactly one filler group on the PE — V-projection t-tiles during pair 0,
then PV(j-1) quarter-groups and QK(j+1) quarter-groups — so neither PE nor
ACT ever starves and HAM stays at full clock.  Input DMAs are split into
prioritized chunks (x halves + w chunk-groups on 4 queues) so the first
exp issues ~11us in instead of ~56us.  One PSUM pool with 3 tags
(s:4 banks, qk:2, o:2) covers all phases — the output projection reuses
the score banks, avoiding a pool-boundary drain.

Matmul operands are fp16 (full-rate PE); accumulation is fp32 in PSUM.
"""

import numpy as np

P = 128
B, N, C = 8, 1024, 768
H, D = 12, 64
SCALE = D ** -0.5  # 0.125
CT = C // P   # 6 contraction chunks
NT = N // P   # 8 sequence tiles
HP = H // 2   # 6 head pairs
NCORES = 8

_cache = {}


def _build_nc():
    import concourse.bass as bass
    import concourse.mybir as mybir
    import concourse.tile as tile
    from concourse import bacc

    f32 = mybir.dt.float32
    f16 = mybir.dt.float16
    Exp = mybir.ActivationFunctionType.Exp

    nc = bacc.Bacc("TRN2", target_bir_lowering=False, debug=False,
                   enable_asserts=False)

    xT = nc.dram_tensor("xT", [C, N], f16, kind="ExternalInput").ap()
    wqT = nc.dram_tensor("wqT", [C, H * D], f16, kind="ExternalInput").ap()
    wkT = nc.dram_tensor("wkT", [C, H * D], f16, kind="ExternalInput").ap()
    wvT = nc.dram_tensor("wvT", [C, H * D], f16, kind="ExternalInput").ap()
    wpT = nc.dram_tensor("wpT", [C, C], f16, kind="ExternalInput").ap()
    bpb = nc.dram_tensor("bpb", [P, C], f32, kind="ExternalInput").ap()
    y = nc.dram_tensor("y", [N, C], f32, kind="ExternalOutput").ap()

    mm = nc.tensor.matmul

    xTr = xT.rearrange("(o p) n -> p o n", p=P)
    wqTr = wqT.rearrange("(o p) f -> p o f", p=P)
    wkTr = wkT.rearrange("(o p) f -> p o f", p=P)
    wvTr = wvT.rearrange("(o p) f -> p o f", p=P)
    wpTr = wpT.rearrange("(o p) f -> p o f", p=P)

    with tile.TileContext(nc) as tc:
        with tc.tile_pool(name="persist", bufs=1) as persist, \
             tc.tile_pool(name="ph1", bufs=1) as ph1, \
             tc.tile_pool(name="ps", bufs=1, space="PSUM") as psp, \
             tc.tile_pool(name="et", bufs=24) as etp, \
             tc.tile_pool(name="sm", bufs=4) as smp, \
             tc.tile_pool(name="outp", bufs=3) as outp:
            qt = persist.tile([P, HP, N], f16)
            kt = persist.tile([P, HP, N], f16)
            vp = persist.tile([P, NT, H, 2 * D], f16)  # [Vh | ones]
            ot = persist.tile([P, HP, N], f16)
            wp_sb = persist.tile([P, CT, C], f16)
            bpb_sb = persist.tile([P, C], f32)

            x_sb = ph1.tile([P, CT, N], f16)
            wq_sb = ph1.tile([P, CT, H * D], f16)
            wk_sb = ph1.tile([P, CT, H * D], f16)
            wv_sb = ph1.tile([P, CT, H * D], f16)

            # --- prioritized chunked input DMAs on 4 queues.  x halves +
            # wq/wk chunk-groups first (QK(0) gates the first exp), wv next
            # (needed by the V-proj fillers from ~13us), wp/bpb last. ---
            nc.sync.dma_start(x_sb[:, :, 0:512], xTr[:, :, 0:512])
            nc.scalar.dma_start(wq_sb[:, 0:3, :], wqTr[:, 0:3, :])
            nc.gpsimd.dma_start(wk_sb[:, 0:3, :], wkTr[:, 0:3, :])
            nc.sync.dma_start(x_sb[:, :, 512:1024], xTr[:, :, 512:1024])
            nc.scalar.dma_start(wq_sb[:, 3:6, :], wqTr[:, 3:6, :])
            nc.gpsimd.dma_start(wv_sb[:, 0:3, :], wvTr[:, 0:3, :])
            nc.gpsimd.dma_start(wk_sb[:, 3:6, :], wkTr[:, 3:6, :])
            nc.gpsimd.dma_start(wv_sb[:, 3:6, :], wvTr[:, 3:6, :])
            nc.gpsimd.dma_start(wp_sb[:], wpTr[:])
            nc.scalar.dma_start(bpb_sb[:], bpb)

            # scratch for PE-warmup matmuls + ACT table preload
            scr = ph1.tile([P, 512], f16)
            scrt = ph1.tile([P, 16], f32)
            nc.vector.memset(scr[:], 0.01)
            nc.vector.memset(vp[:, :, :, D:2 * D], 1.0)
            # preload the exp table set (~2.7us) while input DMAs stream
            nc.scalar.activation(scrt[:], scr[:, 0:16], Exp, scale=1.0)
            # ~4.5us of dummy matmuls: HAM reaches full clock before the
            # first real matmul, whose inputs only land ~5us in
            wps = psp.tile([P, 512], f32, tag="qk", name="wps")
            for i in range(20):
                mm(wps[:], lhsT=scr[:, 0:128], rhs=scr[:],
                   start=(i == 0), stop=(i == 19))

            def emit_qk_group(j, gi):
                # gi 0..3 = (q,nh0), (k,nh0), (q,nh1), (k,nh1)
                w_sb, dst = ((wq_sb, qt), (wk_sb, kt))[gi % 2]
                nh = gi // 2
                ps = psp.tile([P, 512], f32, tag="qk", name="qkps")
                for c in range(CT):
                    mm(ps[:], lhsT=w_sb[:, c, j * P:(j + 1) * P],
                       rhs=x_sb[:, c, nh * 512:(nh + 1) * 512],
                       start=(c == 0), stop=(c == CT - 1))
                nc.vector.tensor_copy(
                    dst[:, j, nh * 512:(nh + 1) * 512], ps[:])

            def emit_v_t(t):
                psa = psp.tile([P, 512], f32, tag="qk", name="psa")
                psb = psp.tile([P, 512], f32, tag="qk", name="psb")
                for c in range(CT):
                    lh = x_sb[:, c, t * P:(t + 1) * P]
                    mm(psa[:], lhsT=lh, rhs=wv_sb[:, c, 0:512],
                       start=(c == 0), stop=(c == CT - 1))
                    mm(psb[:, 0:256], lhsT=lh, rhs=wv_sb[:, c, 512:768],
                       start=(c == 0), stop=(c == CT - 1))
                nc.vector.tensor_copy(
                    vp[:, t, 0:8, 0:D],
                    psa.rearrange("p (h d) -> p h d", d=D))
                nc.vector.tensor_copy(
                    vp[:, t, 8:12, 0:D],
                    psb[:, 0:256].rearrange("p (h d) -> p h d", d=D))

            ets = {}

            def emit_scores_mt(j, mt):
                # One PSUM tile per nh-half holds BOTH heads' scores
                # ([P, h0|h64, 512]), so each exp depends on both row-group
                # matmuls — the scheduler must keep the dual-stream pair
                # together instead of splitting it to unblock ACT early.
                s = {}
                for nh in range(2):
                    s[nh] = psp.tile([P, 2, 512], f32, tag="s",
                                     name=f"s_{nh}")
                    ets[(j, mt, nh)] = etp.tile([P, 2, 512], f16, tag="et",
                                                name=f"et_{nh}")
                for nh in range(2):
                    for hh in range(2):   # adjacent => PE row-group dual
                        r0 = hh * D
                        mm(s[nh][:, hh, :],
                           lhsT=kt[r0:r0 + D, j, mt * P:(mt + 1) * P],
                           rhs=qt[r0:r0 + D, j, nh * 512:(nh + 1) * 512],
                           start=True, stop=True)
                for nh in range(2):
                    nc.scalar.activation(ets[(j, mt, nh)][:], s[nh][:],
                                         Exp, scale=float(SCALE))

            def emit_pv_group(j, hh, nh):
                h = 2 * j + hh
                r0 = hh * D
                pso = psp.tile([P, 512], f32, tag="o", name="pso")
                for mt in range(NT):
                    mm(pso[:], lhsT=vp[:, mt, h],
                       rhs=ets[(j, mt, nh)][:, hh, :],
                       start=(mt == 0), stop=(mt == NT - 1))
                sums = smp.tile([D, 512], f32, tag="sums")
                rec = smp.tile([D, 512], f32, tag="rec")
                nc.vector.tensor_copy(sums[:], pso[D:2 * D, :])
                nc.vector.reciprocal_approx_fast(rec[:], sums[:])
                nc.vector.tensor_mul(
                    ot[r0:r0 + D, j, nh * 512:(nh + 1) * 512],
                    pso[0:D, :], rec[:])

            # ---- prologue: QK for pair 0 (DMA-paced) ----
            for gi in range(4):
                emit_qk_group(0, gi)

            # ---- filler queue: one PE group per score step, in dependency
            # order.  QK(j+1) groups (q=12 MMs, k=12 MMs) always land within
            # pair j's steps; V tiles precede PV(0); PV(j) follows et(j). ----
            # Slot constraints (PE queue is in-order, so violating either
            # deadlocks or stalls the queue):
            #  - pv(jj) group slots must be >= 8*(jj+1): only after pair
            #    jj's own score steps are all emitted may PV(jj) wait on
            #    its exps, else scores behind it starve ACT (circular wait).
            #  - qk(j) group slots must be < 8*j (scores(j,0) reads qt/kt).
            slot_map = {
                0: ("qk", 1, 0), 1: ("qk", 1, 1),
                2: ("v", 0), 3: ("v", 1), 4: ("v", 2), 5: ("v", 3),
                6: ("v", 4), 7: ("v", 5), 8: ("v", 6), 9: ("v", 7),
                10: ("pv", 0, 0, 0), 11: ("pv", 0, 0, 1),
                12: ("pv", 0, 1, 0), 14: ("qk", 2, 0), 15: ("qk", 2, 1),
                16: ("pv", 0, 1, 1),
                18: ("pv", 1, 0, 0), 19: ("pv", 1, 0, 1),
                20: ("pv", 1, 1, 0), 21: ("pv", 1, 1, 1),
                22: ("qk", 3, 0), 23: ("qk", 3, 1),
                24: ("pv", 2, 0, 0), 26: ("pv", 2, 0, 1),
                27: ("pv", 2, 1, 0), 29: ("pv", 2, 1, 1),
                30: ("qk", 4, 0), 31: ("qk", 4, 1),
                32: ("pv", 3, 0, 0), 34: ("pv", 3, 0, 1),
                35: ("pv", 3, 1, 0), 37: ("pv", 3, 1, 1),
                38: ("qk", 5, 0), 39: ("qk", 5, 1),
                41: ("pv", 4, 0, 0), 43: ("pv", 4, 0, 1),
                45: ("pv", 4, 1, 0), 47: ("pv", 4, 1, 1),
            }
            fillers = [slot_map.get(s) for s in range(HP * NT)]
            for s, f in enumerate(fillers):
                if f and f[0] == "pv":
                    assert s >= 8 * (f[1] + 1), (s, f)
                if f and f[0] == "qk":
                    assert s < 8 * f[1], (s, f)

            def emit_filler(f):
                if f is None:
                    return
                if f[0] == "qk":
                    _, j, half = f
                    emit_qk_group(j, 2 * half)      # (q or k, nh0)
                    emit_qk_group(j, 2 * half + 1)  # (q or k, nh1)
                elif f[0] == "v":
                    emit_v_t(f[1])
                else:
                    _, j, hh, nh = f
                    emit_pv_group(j, hh, nh)

            step = 0
            for j in range(HP):
                for mt in range(NT):
                    emit_scores_mt(j, mt)
                    emit_filler(fillers[step])
                    step += 1

            # ---- tail: PV of the last pair, then output projection ----
            for hh in range(2):
                for nh in range(2):
                    emit_pv_group(HP - 1, hh, nh)

            yre = y.rearrange("(t p) f -> t p f", p=P)
            for t in range(NT):
                yps = psp.tile([P, C], f32, tag="s", name="yps")
                for c in range(CT):
                    lh = ot[:, c, t * P:(t + 1) * P]
                    mm(yps[:, 0:512], lhsT=lh, rhs=wp_sb[:, c, 0:512],
                       start=(c == 0), stop=(c == CT - 1))
                    mm(yps[:, 512:768], lhsT=lh, rhs=wp_sb[:, c, 512:768],
                       start=(c == 0), stop=(c == CT - 1))
                ys = outp.tile([P, C], f32, tag="ys")
                nc.vector.tensor_add(ys[:, 0:512], yps[:, 0:512],
                                     bpb_sb[:, 0:512])
                nc.vector.tensor_add(ys[:, 512:768], yps[:, 512:768],
                                     bpb_sb[:, 512:768])
                eng = nc.sync if t % 2 == 0 else nc.scalar
                eng.dma_start(yre[t], ys[:])

    nc.compile()
    return nc


def _get_nc():
    if "nc" not in _cache:
        _cache["nc"] = _build_nc()
    return _cache["nc"]


def _make_in_maps(x, Wq, Wk, Wv, Wp, bp):
    x = np.asarray(x, dtype=np.float32)
    wqT = np.ascontiguousarray(
        np.asarray(Wq, np.float32).reshape(H * D, C).T.astype(np.float16))
    wkT = np.ascontiguousarray(
        np.asarray(Wk, np.float32).reshape(H * D, C).T.astype(np.float16))
    wvT = np.ascontiguousarray(
        np.asarray(Wv, np.float32).reshape(H * D, C).T.astype(np.float16))
    wpT = np.ascontiguousarray(
        np.asarray(Wp, np.float32).T.astype(np.float16))
    bpb = np.ascontiguousarray(
        np.broadcast_to(np.asarray(bp, np.float32), (P, C)))
    in_maps = []
    for b in range(NCORES):
        in_maps.append({
            "xT": np.ascontiguousarray(x[b].T.astype(np.float16)),
            "wqT": wqT, "wkT": wkT, "wvT": wvT, "wpT": wpT, "bpb": bpb,
        })
    return in_maps


def run(x, Wq, Wk, Wv, Wp, bp, trace=False):
    from concourse.bass_utils import run_bass_kernel_spmd
    nc = _get_nc()
    in_maps = _make_in_maps(x, Wq, Wk, Wv, Wp, bp)
    res = run_bass_kernel_spmd(nc, in_maps, list(range(NCORES)), trace=trace)
    out = np.stack([res.results[b]["y"] for b in range(NCORES)])
    return out, res


def kernel(x, Wq, Wk, Wv, Wp, bp):
    out, _ = run(x, Wq, Wk, Wv, Wp, bp)
    return out
